# revision 16
# baseline (speedup 1.0000x reference)
"""Causal self-attention (B=2, S=2048, D=2048, H=16) on 8 TRN2 NeuronCores.

Sharding (data + tensor parallel, per the head-group hint):
  core c -> batch b = c // 4, head group g = c % 4 (heads 4g..4g+3).
  wq/wk/wv are split column-wise per head group (512 cols), wo row-wise
  (512 rows). Each core computes attention for its 4 heads on its batch and
  produces a partial output projection; the host sums the 4 partials per
  batch (the tensor-parallel all-reduce, done at gather time).

Device kernel layout trick: all activations are kept "transposed"
(feature-major) so every matmul consumes operands in their natural layout
and no on-device transpose is ever needed:
  QT[c,s] = wq.T @ x.T          (lhsT=wq,  rhs=xT      — both native)
  KT[c,s] = wk.T @ x.T
  V[s,c]  = x @ wv              (lhsT=xT,  rhs=wv      — both native)
  ST[k,q] = K_h Q_h^T           (lhsT=KT_h, rhs=QT_h)
  PT[k,q] = exp(ST*scale - 4 + causal_mask)             (ACT engine)
  OT[hd,q]= V_h.T @ PT          (lhsT=V_h, rhs=PT)      accumulated in PSUM
  rsum[q] = ones.T @ PT         (softmax denominator, PE ones-matmul)
  out     = (OT/rsum).T @ wo    (lhsT=OT,  rhs=wo)
Compute dtype fp16 (measured matmul rel-err ~3e-4, well under the fp32
envelope gate); softmax statistics and all PSUM accumulation in fp32.
Partial outputs are stored fp16 (halves the output DMA, which overlaps the
next repeat's input loads); the host gather accumulates them in fp32.

Perf notes (measured on HW, 2026-08-08): sustained fp16 matmul issue rate
is ~261 ns per instruction at N=512 moving and barely less at narrower N
(per-instruction floor), independent of PSUM bank rotation or stationary
reuse; N>512 moving is ISA-invalid. So the 1504 matmul instructions here
(768 QKV + 320 score/AV + 160 rsum + 256 out-proj) put the per-repeat PE
floor at ~392 us; hardware-loop slope timing of this kernel measures
~478 us/body. Variants tried and rejected as slower/equal: batched 2-block
exp calls (+75 us), causal-trimmed diagonal matmul widths (neutral —
narrow matmuls don't get cheaper), deeper PSUM score pipelining (neutral).
rsum elimination via q-major scores + ACT accum_out requires a 128x128
probs transpose, which has no cheap path on TRN2 (DVE transposes only
32x32 blocks; PE transpose costs a matmul slot).
"""

import math

import numpy as np

B = 2
S = 2048
D = 2048
H = 16
HD = 128
N_CORES = 8
NH = 4          # heads per core
C = NH * HD     # 512 per-core projection width
P = 128
DO = D // P     # 16 contraction subtiles
SBLK = 512      # matmul moving free dim / PSUM bank
NSB = S // SBLK  # 4 sequence blocks
NKB = S // P     # 16 key blocks
SCALE = 1.0 / math.sqrt(HD)
EBIAS = -4.0    # constant shift inside exp; cancels in softmax ratio
MASK_NEG = -1e9

_STATE = {}


def _build_kernel(repeat=1, hw_loop=False):
    import contextlib

    import concourse.bacc as bacc
    import concourse.mybir as mybir
    import concourse.tile as tile
    from concourse.bass import ts

    F16 = mybir.dt.float16
    F32 = mybir.dt.float32

    nc = bacc.Bacc("TRN2", target_bir_lowering=False, debug=False)

    xt_d = nc.dram_tensor("xt", [D, S], F16, kind="ExternalInput").ap()
    wq_d = nc.dram_tensor("wq", [D, C], F16, kind="ExternalInput").ap()
    wk_d = nc.dram_tensor("wk", [D, C], F16, kind="ExternalInput").ap()
    wv_d = nc.dram_tensor("wv", [D, C], F16, kind="ExternalInput").ap()
    wo_d = nc.dram_tensor("wo", [C, D], F16, kind="ExternalInput").ap()
    out_d = nc.dram_tensor("out", [S, D], F16, kind="ExternalOutput").ap()

    with tile.TileContext(nc) as tc:
        with tc.tile_pool(name="persist", bufs=1) as p_per:
            ot = p_per.tile([P, NH, S], F16)      # normalized attn out^T
            qt = p_per.tile([P, NH, S], F16)
            kt = p_per.tile([P, NH, S], F16)
            v = p_per.tile([P, DO, C], F16)
            masks = p_per.tile([P, NH, SBLK], F32)
            ones = p_per.tile([P, P], F16)
            ebias = p_per.tile([P, 1], F32)

            nc.gpsimd.memset(ones[:], 1.0)
            nc.gpsimd.memset(ebias[:], EBIAS)
            for a in range(4):
                nc.gpsimd.memset(masks[:, a, :], 0.0)
                # keep (j - p - 128a >= 0) i.e. k_global <= q_global
                nc.gpsimd.affine_select(
                    out=masks[:, a, :],
                    in_=masks[:, a, :],
                    compare_op=mybir.AluOpType.is_ge,
                    fill=MASK_NEG,
                    base=-(a * P),
                    channel_multiplier=-1,
                    pattern=[[1, SBLK]],
                )

            # ---------------- Phase 1: QKV projections ----------------
            if hw_loop:
                rep_ctx = tc.For_i(0, repeat)
                rep_iter = [0]
            else:
                rep_ctx = contextlib.nullcontext()
                rep_iter = range(repeat)
            with rep_ctx:
             for _rep in rep_iter:
              # One PSUM ring (tag "w", 4 banks) is shared by phase-1
              # accumulation groups, phase-2 score tiles, and phase-3 output
              # accumulators, plus av(2)+rs(2) = 8 banks total with NO pool
              # open/close transitions between phases.  Small SBUF work pools
              # are allocated BEFORE the big xt pool so they never land on
              # xt addresses (which would serialize phase 2 behind the last
              # phase-1 xt read).
              with tc.tile_pool(name="p2w", bufs=4) as p2w, \
                   tc.tile_pool(name="p2stat", bufs=2) as p2stat, \
                   tc.tile_pool(name="p3stage", bufs=4) as p3stage, \
                   tc.tile_pool(name="psw", bufs=4, space="PSUM") as psw, \
                   tc.tile_pool(name="ps_av", bufs=2, space="PSUM") as ps_av, \
                   tc.tile_pool(name="ps_rs", bufs=2, space="PSUM") as ps_rs:
               # ---------------- Phase 1: QKV projections ----------------
               with tc.tile_pool(name="xw", bufs=1) as p_xw:
                  xt_r = xt_d.rearrange("(do p) s -> do p s", p=P)
                  xts = []
                  for do in range(DO):
                      t = p_xw.tile([P, S], F16, tag=f"xt{do}", name=f"xt{do}")
                      # alternate the two HWDGE engines for queue parallelism
                      eng = nc.sync if do % 2 == 0 else nc.scalar
                      eng.dma_start(t[:], xt_r[do])
                      xts.append(t)
                  wq_sb = p_xw.tile([P, DO, C], F16, tag="wq")
                  wk_sb = p_xw.tile([P, DO, C], F16, tag="wk")
                  wv_sb = p_xw.tile([P, DO, C], F16, tag="wv")
                  # chunk weight loads by 4 d-subtiles so the first matmul
                  # rounds start after 512 KB, not after the full 2 MB
                  wq_r = wq_d.rearrange("(do p) c -> p do c", p=P)
                  wk_r = wk_d.rearrange("(do p) c -> p do c", p=P)
                  wv_r = wv_d.rearrange("(do p) c -> p do c", p=P)
                  for dc in range(0, DO, 4):
                      sl = slice(dc, dc + 4)
                      nc.scalar.dma_start(wq_sb[:, sl, :], wq_r[:, sl, :])
                      nc.sync.dma_start(wk_sb[:, sl, :], wk_r[:, sl, :])
                      nc.scalar.dma_start(wv_sb[:, sl, :], wv_r[:, sl, :])

                  # 48 accumulation groups, st-major with V interleaved so
                  # the tiles phase 2 needs first (st=0 QT/KT rows + low-kb
                  # V rows) are produced first, and each xt subtile is
                  # consumed right after its DMA completes.
                  groups = []
                  for st in range(NSB):
                      for ct in range(NH):
                          groups.append(("q", ct, st))
                          groups.append(("k", ct, st))
                      for sv in range(4 * st, 4 * st + 4):
                          groups.append(("v", sv, 0))

                  GCHUNK = 2
                  for gstart in range(0, len(groups), GCHUNK):
                      chunk = groups[gstart:gstart + GCHUNK]
                      psums = []
                      for kind, i0, i1 in chunk:
                          psums.append(psw.tile([P, SBLK], F32, tag="w", name="p1ps"))
                      for do in range(DO):
                          for gi, (kind, i0, i1) in enumerate(chunk):
                              first = do == 0
                              last = do == DO - 1
                              if kind == "q":
                                  nc.tensor.matmul(
                                      psums[gi][:],
                                      wq_sb[:, do, ts(i0, P)],
                                      xts[do][:, ts(i1, SBLK)],
                                      start=first, stop=last)
                              elif kind == "k":
                                  nc.tensor.matmul(
                                      psums[gi][:],
                                      wk_sb[:, do, ts(i0, P)],
                                      xts[do][:, ts(i1, SBLK)],
                                      start=first, stop=last)
                              else:
                                  nc.tensor.matmul(
                                      psums[gi][:],
                                      xts[do][:, ts(i0, P)],
                                      wv_sb[:, do, :],
                                      start=first, stop=last)
                      for gi, (kind, i0, i1) in enumerate(chunk):
                          if kind == "q":
                              nc.any.tensor_copy(qt[:, i0, ts(i1, SBLK)], psums[gi][:])
                          elif kind == "k":
                              nc.any.tensor_copy(kt[:, i0, ts(i1, SBLK)], psums[gi][:])
                          else:
                              nc.any.tensor_copy(v[:, i0, :], psums[gi][:])

               # ------- Phases 2+3 fused: attention + output per q-block ----
               with tc.tile_pool(name="p3w", bufs=1) as p3w:
                  # wo lands in the space freed by the xt/weight pool
                  wo_sb = p3w.tile([P, NH, D], F16, tag="wo")
                  nc.scalar.dma_start(wo_sb[:], wo_d.rearrange("(cs p) d -> p cs d", p=P))
                  for qb in range(NSB):
                      nkb = 4 * (qb + 1)  # causal: only key blocks <= q block
                      for h in range(NH):
                          av = ps_av.tile([P, SBLK], F32, tag="av")
                          rs = ps_rs.tile([P, SBLK], F32, tag="rs")
                          for kb in range(nkb):
                              sc = psw.tile([P, SBLK], F32, tag="w", name="sc")
                              nc.tensor.matmul(
                                  sc[:],
                                  kt[:, h, ts(kb, P)],
                                  qt[:, h, ts(qb, SBLK)],
                                  start=True, stop=True)
                              if kb >= nkb - 4:
                                  a = kb - 4 * qb
                                  tmp = p2w.tile([P, SBLK], F32, tag="msk")
                                  nc.vector.tensor_add(tmp[:], sc[:], masks[:, a, :])
                                  src = tmp
                              else:
                                  src = sc
                              probs = p2w.tile([P, SBLK], F16, tag="probs")
                              nc.scalar.activation(
                                  probs[:], src[:],
                                  mybir.ActivationFunctionType.Exp,
                                  bias=ebias[:], scale=SCALE)
                              nc.tensor.matmul(
                                  av[:],
                                  v[:, kb, ts(h, P)],
                                  probs[:],
                                  start=(kb == 0), stop=(kb == nkb - 1))
                              nc.tensor.matmul(
                                  rs[:],
                                  ones[:],
                                  probs[:],
                                  start=(kb == 0), stop=(kb == nkb - 1))
                          rcp = p2stat.tile([P, SBLK], F32, tag="rcp")
                          nc.vector.reciprocal(rcp[:], rs[:])
                          nc.vector.tensor_tensor(
                              ot[:, h, ts(qb, SBLK)], av[:], rcp[:],
                              op=mybir.AluOpType.mult)
                      # output projection for this q-block: streams out while
                      # the next q-block's attention runs
                      for sl in range(NSB):
                          so = 4 * qb + sl
                          for no in range(NSB):
                              po = psw.tile([P, SBLK], F32, tag="w", name="po")
                              for cs in range(NH):
                                  nc.tensor.matmul(
                                      po[:],
                                      ot[:, cs, ts(so, P)],
                                      wo_sb[:, cs, ts(no, SBLK)],
                                      start=(cs == 0), stop=(cs == NH - 1))
                              stage = p3stage.tile([P, SBLK], F16, tag="st")
                              nc.any.tensor_copy(stage[:], po[:])
                              eng = nc.sync if (so * NSB + no) % 2 == 0 else nc.scalar
                              eng.dma_start(
                                  out_d[ts(so, P), ts(no, SBLK)], stage[:])

    nc.compile()
    return nc


def _shard_inputs(x, wq, wk, wv, wo):
    in_maps = []
    for c in range(N_CORES):
        b, g = divmod(c, NH)
        cols = slice(g * C, (g + 1) * C)
        in_maps.append({
            "xt": np.ascontiguousarray(x[b].T).astype(np.float16),
            "wq": wq[:, cols].astype(np.float16),
            "wk": wk[:, cols].astype(np.float16),
            "wv": wv[:, cols].astype(np.float16),
            "wo": np.ascontiguousarray(wo[cols, :]).astype(np.float16),
        })
    return in_maps


def kernel(x, wq, wk, wv, wo):
    from concourse.bass_utils import run_bass_kernel_spmd

    if "nc" not in _STATE:
        _STATE["nc"] = _build_kernel()
    nc = _STATE["nc"]

    in_maps = _shard_inputs(
        np.asarray(x), np.asarray(wq), np.asarray(wk),
        np.asarray(wv), np.asarray(wo))
    res = run_bass_kernel_spmd(nc, in_maps, core_ids=list(range(N_CORES)))
    out = np.zeros((B, S, D), dtype=np.float32)
    for c in range(N_CORES):
        b = c // NH
        out[b] += res.results[c]["out"]
    return out

